# revision 9
# baseline (speedup 1.0000x reference)
"""Trainium2 Bass kernel for nn_BrainInspiredRNN (GRU-like RNN, low-rank recurrent weights).

Strategy (data-parallel over 8 NeuronCores, batch sharded B=4096 -> 512/core):
  - Host precomputes fused weight matrices:
      Wfull [32, 98] : columns = [Vr@Ur.T | Vz@Uz.T | Vn@Un.T | Wout.T]
      WiExt [3, 98]  : columns = [Wir.T | Wiz.T | 0 | 0]
      Win3  [3, 32]  : Win.T
    and per-core transposed input xt [Tpad, 3, 512] (time-major, channel on
    partitions) plus h0T [32, 512].
  - Device scan, h kept h-major [32, 512] in SBUF.  Per step:
      psumG[96,512]  = Wfull.T @ h  (+ WiExt.T @ x_t)      (TensorE, strips 0/64)
      rz    = sigmoid(psumG[0:64] + b_rz)                  (ScalarE, bias fold)
      m2    = (psumG[64:96] + b_hn) * r                    (VectorE STT)
      psumN = Win3.T @ x_t + I32 @ m2                      (TensorE accumulate)
      n     = tanh(psumN + b_in)                           (ScalarE, bias fold)
      h'    = n + z * (h - n)                              (VectorE x3)
      h' -> DRAM hscr[t]                                   (DMA, SBUF src)
  - h_t is DMA'd to DRAM each step (no extra compute); host does the tiny
    readout y = hs @ Wout.T + b_out (134M MACs, BLAS).
"""

import os
import sys

import numpy as np

for _p in ("/opt/trn_rl_repo", "/root/.axon_site/_ro/trn_rl_repo"):
    if os.path.isdir(_p) and _p not in sys.path:
        sys.path.insert(0, _p)

import concourse.bacc as bacc
import concourse.bass as bass
import concourse.mybir as mybir
import concourse.tile as tile
from concourse.bass_utils import run_bass_kernel_spmd

B, T, NIN, H, NOUT = 4096, 512, 3, 32, 2
NCORES = 8
BS = B // NCORES          # batch per core
CHUNK = 16                # time steps per x-stage DMA chunk
NSTEP = T
TPAD = ((NSTEP + CHUNK - 1) // CHUNK) * CHUNK
FP32 = mybir.dt.float32

_nc_cache = {}


def _build_program(nsteps=NSTEP):
    key = ("nc", nsteps)
    if key in _nc_cache:
        return _nc_cache[key]

    nc = bacc.Bacc()

    xt_d = nc.declare_dram_parameter("xt", [TPAD, NIN, BS], FP32, isOutput=False)
    h0t_d = nc.declare_dram_parameter("h0t", [H, BS], FP32, isOutput=False)
    # all small constants packed into one blob -> one DMA -> one sem wait
    blob_d = nc.declare_dram_parameter("blob", [128, 259], FP32, isOutput=False)
    hscr_d = nc.declare_dram_parameter("hscr", [nsteps, H, BS], FP32, isOutput=True)

    SIG = mybir.ActivationFunctionType.Sigmoid
    TANH = mybir.ActivationFunctionType.Tanh
    MULT = mybir.AluOpType.mult
    ADD = mybir.AluOpType.add
    SUB = mybir.AluOpType.subtract

    with tile.TileContext(nc) as tc:
        with (
            tc.tile_pool(name="const", bufs=1) as cpool,
            tc.tile_pool(name="xstage", bufs=2) as xpool,
            tc.tile_pool(name="hpool", bufs=3) as hpool,
            tc.tile_pool(name="rzpool", bufs=2) as rzpool,
            tc.tile_pool(name="tmp", bufs=2) as tpool,
            tc.tile_pool(name="psg", bufs=4, space="PSUM") as pgpool,
            tc.tile_pool(name="psn", bufs=4, space="PSUM") as pnpool,
        ):
            # constants / weights: single blob tile, sliced
            blob = cpool.tile([128, 259], FP32, tag="blob")
            nc.sync.dma_start(blob[:], blob_d[:])
            wf = blob[0:H, 0:96]
            eye = blob[0:H, 96:128]
            brz = blob[0:2 * H, 256:257]
            bhn = blob[0:H, 257:258]
            bin_ = blob[0:H, 258:259]

            h_prev = hpool.tile([H, BS], FP32, tag="h")
            nc.sync.dma_start(h_prev[:], h0t_d[:])

            xs = None
            for s in range(nsteps):
                toff = s % CHUNK
                if toff == 0:
                    xs = xpool.tile([NIN, CHUNK * BS], FP32, tag="xs")
                    src = xt_d[s:s + CHUNK].rearrange("t c b -> c t b")
                    dst = xs[:, :].rearrange("c (t b) -> c t b", t=CHUNK)
                    nc.sync.dma_start(dst, src)

                xcur = xs[0:NIN, toff * BS:(toff + 1) * BS]

                pg = pgpool.tile([96, BS], FP32, tag="pg")
                nc.tensor.matmul(pg[:], wf, h_prev[:], start=True, stop=False)
                nc.tensor.matmul(pg[:], blob[0:NIN, 128:224], xcur,
                                 start=False, stop=True)

                pn = pnpool.tile([H, BS], FP32, tag="pn")
                nc.tensor.matmul(pn[:], blob[0:NIN, 224:256], xcur,
                                 start=True, stop=False)

                rz = rzpool.tile([2 * H, BS], FP32, tag="rz")
                nc.scalar.activation(rz[:], pg[0:64, :], SIG, bias=brz)

                m2 = tpool.tile([H, BS], FP32, tag="m2")
                nc.vector.scalar_tensor_tensor(
                    m2[:], pg[64:96, :], bhn, rz[0:H, :], op0=ADD, op1=MULT)

                nc.tensor.matmul(pn[:], eye, m2[:], start=False, stop=True)

                nn = tpool.tile([H, BS], FP32, tag="nn")
                nc.scalar.activation(nn[:], pn[:], TANH, bias=bin_)

                # dd parked at partitions 32:64 so the zd tensor_tensor sees
                # equal SBUF base partitions (walrus samePartitionsAll rule)
                dd = tpool.tile([2 * H, BS], FP32, tag="dd")
                nc.vector.tensor_tensor(dd[H:2 * H, :], h_prev[:], nn[:], op=SUB)

                zd = tpool.tile([H, BS], FP32, tag="zd")
                nc.vector.tensor_tensor(zd[:], rz[H:2 * H, :], dd[H:2 * H, :],
                                        op=MULT)

                h_new = hpool.tile([H, BS], FP32, tag="h")
                nc.vector.tensor_tensor(h_new[:], nn[:], zd[:], op=ADD)
                nc.sync.dma_start(hscr_d[s], h_new[:])
                h_prev = h_new

    if not nc.is_finalized():
        nc.finalize()   # Bacc: runs wait-legalization + register allocation
    _nc_cache[key] = nc
    return nc


def _prep_inputs(x, h0, Wir, b_ir, Wiz, b_iz, Win, b_in,
                 Ur, Vr, b_hr, Uz, Vz, b_hz, Un, Vn, b_hn, Wout, b_out):
    f = np.float32
    wfull = np.concatenate(
        [Vr @ Ur.T, Vz @ Uz.T, Vn @ Un.T], axis=1).astype(f)
    wiext = np.zeros((NIN, 96), f)
    wiext[:, 0:H] = Wir.T
    wiext[:, H:2 * H] = Wiz.T
    win3 = np.ascontiguousarray(Win.T).astype(f)
    eye = np.eye(H, dtype=f)
    blob = np.zeros((128, 259), f)
    blob[0:H, 0:96] = wfull
    blob[0:H, 96:128] = eye
    blob[0:NIN, 128:224] = wiext
    blob[0:NIN, 224:256] = win3
    blob[0:2 * H, 256] = np.concatenate([b_ir + b_hr, b_iz + b_hz])
    blob[0:H, 257] = b_hn
    blob[0:H, 258] = b_in

    # xt: [NCORES, TPAD, NIN, BS]; transposed, zero-padded past T
    xt = np.zeros((NCORES, TPAD, NIN, BS), f)
    xt[:, :T] = np.ascontiguousarray(
        x.reshape(NCORES, BS, T, NIN).transpose(0, 2, 3, 1))
    h0t = np.ascontiguousarray(
        h0.reshape(NCORES, BS, H).transpose(0, 2, 1)).astype(f)

    in_maps = []
    for i in range(NCORES):
        in_maps.append({"xt": xt[i], "h0t": h0t[i], "blob": blob})
    return in_maps, Wout.astype(f), b_out.astype(f)


def _run(inputs, trace=False, nsteps=NSTEP, **kw):
    nc = _build_program(nsteps)
    in_maps, Wout, b_out = _prep_inputs(**inputs)
    res = run_bass_kernel_spmd(nc, in_maps, list(range(NCORES)),
                               trace=trace, **kw)
    outs = []
    for i in range(NCORES):
        hscr = np.asarray(res.results[i]["hscr"])      # [T, H, BS]
        # y[b, t, o] = sum_k hscr[t, k, b] * Wout[o, k] + b_out[o]
        yi = np.einsum("tkb,ok->bto", hscr, Wout, optimize=True)
        outs.append(yi)
    y = np.concatenate(outs, axis=0) + b_out[None, None, :]
    return y.astype(np.float32), res


def kernel(**inputs):
    inputs = {k: np.asarray(v) for k, v in inputs.items()}
    y, _ = _run(inputs, trace=False)
    return y
